# revision 18
# baseline (speedup 1.0000x reference)
"""Gated TCN layer (fully conditioned) as a Bass/Tile kernel on 8 NeuronCores.

Reference computation (per sample b):
    kern = (c @ adapter_w + adapter_b).reshape(2*CH, CH, K)
    y    = dilated causal conv of x with per-sample kern (K=3, dil=4)
    y   += (c @ bias_w + bias_b)[:, None]
    z    = tanh(y[:CH]) * sigmoid(y[CH:])
    out  = resi_w @ z + resi_b + x
Returns (out, z).

Sharding: data-parallel over batch, 2 samples per core. Both samples are
packed onto the 128 SBUF partitions as (b, channel) so every matmul /
activation / DVE op runs with full 128-partition tiles:
  - conv matmuls use block-diagonal per-tap weights [ (b,i) x (b,o) ],
    one PSUM tile per gate half => tanh/sigmoid each run on [128, 1024].
  - dilation taps are plain column offsets into one [128, PAD+T] x buffer.
  - residual 1x1 conv uses block-diagonal resi_w.T, software-pipelined one
    block behind the conv so its z dependency never stalls the PE.
The adapter output is accumulated into a single PSUM bank: chunk c uses
lhsT = cTpad[:, 128-2c:130] (a zero-padded c tile) so its [2, 512] result
lands at partitions [2c, 2c+2) while accumulating zeros elsewhere - all
48 chunks drain with ONE [96, 512] copy instead of 48 thin ones.
All matmul operands are bf16 (PSUM accumulation stays fp32).

DMA routing exploits the per-ring FIFO of the two HWDGE descriptor
generators: the adapter weights (kern critical path) go first, split
across both rings; x behind them; stores split across rings so they
never queue behind input transfers. Outputs are stored bf16 in a
block-major DRAM layout ([8, 128, 2048]); the host converts to fp32,
adds resi_b, and unpacks.
"""

import numpy as np
import ml_dtypes

from concourse import bacc, mybir, tile
from concourse.bass_utils import run_bass_kernel_spmd

K = 3
DIL = 4
CH = 64
COND = 128
B, T = 16, 16384
NCORES = 8
BL = B // NCORES          # samples per core
PAD = (K - 1) * DIL       # causal left pad = 8
XW = PAD + T              # x buffer width
UW = 1024                 # block width (2 PSUM banks)
NJ = T // UW              # 16 blocks
SC = 2048                 # store chunk width (cols)
NSC = T // SC             # 8 store chunks
F = K * CH * 2 * CH       # 24576 adapter columns
NCH = F // 512            # 48 adapter chunks
AWC = F // 4              # aw dma chunk cols (6144)

F32 = mybir.dt.float32
BF16 = mybir.dt.bfloat16
AF = mybir.ActivationFunctionType

# Set by test.py to capture a profile; harness path leaves these alone.
TRACE = False
LAST_RESULTS = None

_NC = None


def _build():
    nc = bacc.Bacc("TRN2", target_bir_lowering=False, debug=False)

    x_d = nc.dram_tensor("x_up", [128, XW], BF16, kind="ExternalInput")
    aw_d = nc.dram_tensor("aw_r", [COND, F], BF16, kind="ExternalInput")
    cp_d = nc.dram_tensor("cTpad", [COND, 130], BF16, kind="ExternalInput")
    cT_d = nc.dram_tensor("cT", [COND, BL], BF16, kind="ExternalInput")
    bw_d = nc.dram_tensor("bw", [COND, 2 * CH], BF16, kind="ExternalInput")
    bb_d = nc.dram_tensor("bb", [1, 2 * CH], BF16, kind="ExternalInput")
    ab_d = nc.dram_tensor("ab6", [128, 6 * 128], F32, kind="ExternalInput")
    riw_d = nc.dram_tensor("riw2", [128, 128], BF16, kind="ExternalInput")
    z_d = nc.dram_tensor("z_d", [NSC, 128, SC], BF16, kind="ExternalOutput")
    o_d = nc.dram_tensor("o_d", [NSC, 128, SC], BF16, kind="ExternalOutput")

    with tile.TileContext(nc) as tc:
        with (
            tc.tile_pool(name="const", bufs=1) as constp,
            tc.tile_pool(name="xpool", bufs=1) as xpool,
            tc.tile_pool(name="awp", bufs=1) as awp,
        ):
            # --- input DMAs: aw (kern critical path) first, split across
            # both HWDGE rings in reverse chunk order; x behind it.
            cp_sb = constp.tile([COND, 130], BF16)
            nc.sync.dma_start(cp_sb[:, :], cp_d[:, :])
            awt = []
            for q in range(4):
                t = awp.tile([COND, AWC], BF16, tag=f"aw{q}")
                awt.append(t)
            for q, eng in ((3, nc.sync), (2, nc.scalar), (1, nc.sync),
                           (0, nc.scalar)):
                eng.dma_start(awt[q][:, :], aw_d[:, q * AWC:(q + 1) * AWC])
            ab_sb = constp.tile([128, 6 * 128], F32)
            nc.scalar.dma_start(ab_sb[:, :], ab_d[:, :])
            cT_sb = constp.tile([COND, BL], BF16)
            nc.scalar.dma_start(cT_sb[:, :], cT_d[:, :])
            bw_sb = constp.tile([COND, 2 * CH], BF16)
            nc.scalar.dma_start(bw_sb[:, :], bw_d[:, :])
            bb_sb = constp.tile([1, 2 * CH], BF16)
            nc.scalar.dma_start(bb_sb[:, :], bb_d[:, :])
            riw_sb = constp.tile([128, 128], BF16)
            nc.scalar.dma_start(riw_sb[:, :], riw_d[:, :])
            xbuf = xpool.tile([128, XW], BF16)
            for q, eng in ((0, nc.sync), (1, nc.scalar), (2, nc.sync),
                           (3, nc.scalar)):
                c0, c1 = q * (XW // 4), (q + 1) * (XW // 4)
                eng.dma_start(xbuf[:, c0:c1], x_d[:, c0:c1])
            ones_sb = constp.tile([1, BL], BF16)
            nc.vector.memset(ones_sb[:, :], 1.0)

            stage2 = constp.tile([112, 512], BF16, name="stage2")
            kst = constp.tile([128, 6 * 128], BF16, name="kst")
            kbf = constp.tile([128, 6 * 128], BF16, name="kbf")
            nc.vector.memset(kst[:, :], 0.0)
            bias_tmp = constp.tile([2 * CH, BL], F32)
            b1 = constp.tile([128, 1], F32)
            b2 = constp.tile([128, 1], F32)

            # ---------------- phase A: adapter ----------
            # chunk c (descending): lhsT = cTpad[:, 128-2c':130] puts the
            # [2, 512] result at partitions [2c', 2c'+2), zeros accumulate
            # elsewhere. Two PSUM banks so bank A (chunks 24-47, kst tiles
            # 3-5, from the first-loaded aw) drains and scatters while bank
            # B (chunks 0-23) is still accumulating.
            def scatter_tile(t):
                for b in range(BL):
                    base, tl = (64, t) if t < 3 else (0, t - 3)
                    src = stage2[base + 16 * tl + b:
                                 base + 16 * tl + b + 15:2, :]
                    dst = kst[CH * b:CH * (b + 1),
                              t * 128 + CH * b:t * 128 + CH * b + CH]
                    eng = nc.sync if (t * BL + b) % 2 == 0 else nc.scalar
                    eng.dma_start(dst, src)

            def add_tile(t):
                nc.vector.tensor_add(
                    kbf[:, t * 128:(t + 1) * 128],
                    kst[:, t * 128:(t + 1) * 128],
                    ab_sb[:, t * 128:(t + 1) * 128])

            with (
                tc.tile_pool(name="apsum", bufs=1, space="PSUM") as apsum,
                tc.tile_pool(name="bpsum", bufs=1, space="PSUM") as bpsum,
            ):
                psA = apsum.tile([NCH, 512], F32, name="psA", tag="a")
                psB = apsum.tile([NCH, 512], F32, name="psB", tag="b")
                for c in range(NCH - 1, 23, -1):
                    cc, cl = c * 512, c - 24
                    nc.tensor.matmul(
                        psA[0:2 * cl + 2, :],
                        cp_sb[:, 128 - 2 * cl:130],
                        awt[cc // AWC][:, cc % AWC:cc % AWC + 512],
                        start=(c == NCH - 1), stop=(c == 24),
                    )
                nc.scalar.activation(stage2[0:48, :], psA[:, :], AF.Copy)
                for t in (3, 4, 5):
                    scatter_tile(t)
                    add_tile(t)
                for c in range(23, -1, -1):
                    cc = c * 512
                    nc.tensor.matmul(
                        psB[0:2 * c + 2, :],
                        cp_sb[:, 128 - 2 * c:130],
                        awt[cc // AWC][:, cc % AWC:cc % AWC + 512],
                        start=(c == 23), stop=(c == 0),
                    )
                nc.vector.tensor_copy(stage2[64:112, :], psB[:, :])
                for t in (0, 1, 2):
                    scatter_tile(t)
                    add_tile(t)

                # conditioned bias (after adapter on the tensor queue)
                pb = bpsum.tile([2 * CH, BL], F32, name="pb", tag="c")
                nc.tensor.matmul(pb[:, :], bw_sb[:, :], cT_sb[:, :],
                                 start=True, stop=False)
                nc.tensor.matmul(pb[:, :], bb_sb[:, :], ones_sb[:, :],
                                 start=False, stop=True)
                nc.vector.tensor_copy(bias_tmp[:, :], pb[:, :])

            # bias rearrange [o2, b] -> packed [(b,o), half]
            for b in range(BL):
                nc.sync.dma_start(b1[CH * b:CH * (b + 1), :],
                                  bias_tmp[0:CH, b:b + 1])
                nc.sync.dma_start(b2[CH * b:CH * (b + 1), :],
                                  bias_tmp[CH:2 * CH, b:b + 1])

            # ---------------- phase B: conv + gate + residual ----------
            with (
                tc.tile_pool(name="p1", bufs=2, space="PSUM") as p1,
                tc.tile_pool(name="p2", bufs=1, space="PSUM") as p2,
                tc.tile_pool(name="pres", bufs=1, space="PSUM") as pres,
                tc.tile_pool(name="work", bufs=4) as workp,
                tc.tile_pool(name="zc", bufs=2) as zcp,
                tc.tile_pool(name="oc", bufs=2) as ocp,
            ):
                zts = {}
                ots = {}

                def emit_resid(i):
                    # residual + out assembly for block i (one block behind)
                    ci = i // 2
                    si = (i % 2) * UW
                    if i % 2 == 0:
                        ots[ci] = ocp.tile([128, SC], BF16, tag="o", name="och")
                    po = pres.tile([128, UW], F32, tag="po")
                    for h in range(2):
                        nc.tensor.matmul(
                            po[:, h * 512:(h + 1) * 512],
                            riw_sb[:, :],
                            zts[ci][:, si + h * 512:si + (h + 1) * 512],
                            start=True, stop=True,
                        )
                    nc.vector.tensor_add(
                        ots[ci][:, si:si + UW], po[:, :],
                        xbuf[:, i * UW + PAD:i * UW + PAD + UW])
                    if i % 2 == 1:
                        nc.scalar.dma_start(z_d[ci], zts[ci][:, :])
                        nc.sync.dma_start(o_d[ci], ots[ci][:, :])

                for j in range(NJ):
                    cb = j * UW
                    cj = j // 2
                    if j % 2 == 0:
                        zts[cj] = zcp.tile([128, SC], BF16, tag="z", name="zch")
                    sl = (j % 2) * UW
                    py1 = p1.tile([128, UW], F32, tag="py1")
                    py2 = p2.tile([128, UW], F32, tag="py2")
                    for h in range(2):
                        for k in range(K):
                            nc.tensor.matmul(
                                py1[:, h * 512:(h + 1) * 512],
                                kbf[:, (k * 2 + 0) * 128:(k * 2 + 1) * 128],
                                xbuf[:, cb + h * 512 + k * DIL:
                                     cb + h * 512 + k * DIL + 512],
                                start=(k == 0), stop=(k == K - 1),
                            )
                    for h in range(2):
                        for k in range(K):
                            nc.tensor.matmul(
                                py2[:, h * 512:(h + 1) * 512],
                                kbf[:, (k * 2 + 1) * 128:(k * 2 + 2) * 128],
                                xbuf[:, cb + h * 512 + k * DIL:
                                     cb + h * 512 + k * DIL + 512],
                                start=(k == 0), stop=(k == K - 1),
                            )
                    if j >= 1:
                        emit_resid(j - 1)
                    ta = workp.tile([128, UW], BF16, tag="ta")
                    nc.scalar.activation(ta[:, :], py1[:, :], AF.Tanh,
                                         bias=b1[:, 0:1])
                    ts = workp.tile([128, UW], BF16, tag="ts")
                    nc.scalar.activation(ts[:, :], py2[:, :], AF.Sigmoid,
                                         bias=b2[:, 0:1])
                    nc.vector.tensor_mul(zts[cj][:, sl:sl + UW],
                                         ta[:, :], ts[:, :])
                emit_resid(NJ - 1)

    nc.compile()
    return nc


def get_nc():
    global _NC
    if _NC is None:
        _NC = _build()
    return _NC


def make_in_maps(inputs):
    x = np.asarray(inputs["x"], np.float32)
    c = np.asarray(inputs["c"], np.float32)
    aw = np.asarray(inputs["adapter_w"], np.float32)
    ab = np.asarray(inputs["adapter_b"], np.float32)
    bw = np.asarray(inputs["bias_w"], np.float32)
    bb = np.asarray(inputs["bias_b"], np.float32).reshape(1, 2 * CH)
    rw = np.asarray(inputs["resi_w"], np.float32)

    bf = ml_dtypes.bfloat16
    # adapter cols (o2,i,k) -> (k, h, i, o): o2 = h*64+o
    aw5 = aw.reshape(COND, 2, CH, CH, K)           # [cond, h, o, i, k]
    aw_r = np.ascontiguousarray(
        aw5.transpose(0, 4, 1, 3, 2).reshape(COND, F).astype(bf))
    ab5 = ab.reshape(2, CH, CH, K)                 # [h, o, i, k]
    ab6 = np.zeros((128, 6 * 128), np.float32)
    for k in range(K):
        for h in range(2):
            t = k * 2 + h
            blk = ab5[h, :, :, k].T                # [i, o]
            for b in range(BL):
                ab6[CH * b:CH * (b + 1),
                    t * 128 + CH * b:t * 128 + CH * b + CH] = blk
    riw2 = np.zeros((128, 128), np.float32)
    for b in range(BL):
        riw2[CH * b:CH * (b + 1), CH * b:CH * (b + 1)] = rw.T
    riw2 = np.ascontiguousarray(riw2.astype(bf))

    in_maps = []
    for m in range(NCORES):
        sl = slice(BL * m, BL * (m + 1))
        xs = x[sl]                                 # [2, 64, T]
        x_up = np.zeros((128, XW), bf)
        for b in range(BL):
            x_up[CH * b:CH * (b + 1), PAD:] = xs[b].astype(bf)
        cpad = np.zeros((COND, 130), np.float32)
        cpad[:, 128:130] = c[sl].T
        in_maps.append(
            {
                "x_up": x_up,
                "aw_r": aw_r,
                "cTpad": np.ascontiguousarray(cpad.astype(bf)),
                "cT": np.ascontiguousarray(c[sl].T.astype(bf)),
                "bw": np.ascontiguousarray(bw.astype(bf)),
                "bb": np.ascontiguousarray(bb.astype(bf)),
                "ab6": ab6,
                "riw2": riw2,
            }
        )
    return in_maps


def kernel(**inputs):
    global LAST_RESULTS
    nc = get_nc()
    in_maps = make_in_maps(inputs)
    res = run_bass_kernel_spmd(
        nc, in_maps, list(range(NCORES)), trace=TRACE
    )
    LAST_RESULTS = res
    rb = np.asarray(inputs["resi_b"], np.float32).reshape(1, CH, 1)
    out = np.empty((B, CH, T), np.float32)
    z = np.empty((B, CH, T), np.float32)
    for m in range(NCORES):
        # [NSC, 128, SC] -> [128, T] -> [2, 64, T]
        zd = np.asarray(res.results[m]["z_d"]).transpose(1, 0, 2)
        od = np.asarray(res.results[m]["o_d"]).transpose(1, 0, 2)
        zd = zd.reshape(BL, CH, T).astype(np.float32)
        od = od.reshape(BL, CH, T).astype(np.float32)
        z[BL * m:BL * (m + 1)] = zd
        out[BL * m:BL * (m + 1)] = od + rb
    return out, z
